# revision 24
# baseline (speedup 1.0000x reference)
"""FourierKAN layer (N=16384, I=128, O=128, G=16) on 8 Trainium2 NeuronCores.

Design (data-parallel over N, 2048 rows/core):
 - Basis of 33 fp16 tiles spanning harmonics {1, cos gx, sin gx, g<=16}:
   directs {1,4} via fp32 round-constant range reduction + ACT Sin;
   pure cosine carriers c4, c8 via Square+affine-Copy; sin-side chain and
   20 two-factor products (affine q-carriers; host LS absorbs the mixing).
 - Host solves exact LS weights W[b,i,o] (fp64) mapping basis -> amplitudes,
   centered over o (folds LayerNorm mean-subtraction into the weights).
 - Op DAG runs per 1024-col half (balances DVE fixed overhead against
   pipeline latency); each produced tile immediately feeds its two 512-col
   PSUM banks' accumulating matmuls. The last 4 tiles are produced per
   512-block to stagger bank completion and overlap the output DMAs.
 - Pool/GpSimd does no elementwise compute (its tensor ops run ~3.3x slower
   AND steal DVE SBUF ports); PE warmup matmuls counter the HAM clock gate.
 - Device returns centered pre-LN y [O, N]; host applies the cheap
   normalization y*rsqrt(mean(y^2)+eps)*gamma+beta and the transpose.
"""
import sys

sys.path.insert(0, "/opt/trn_rl_repo")

import numpy as np

import concourse.bass as bass
import concourse.mybir as mybir
from concourse.tile import TileContext
from contextlib import ExitStack

import bass_rust
from concourse import tile as _tile


def _patched_drain_and_barrier(self, tick_clock, wait_clock):
    nc = self.nc
    gc = tick_clock.global_clock
    n = len(gc)
    for p in range(n):
        if gc[p] > 0:
            vc = bass_rust.VectorClock([0] * n)
            vc.require_at_least(p, gc[p])
            nop = nc.sync.nop(hint="drain_wait_carrier", nofuse=True)
            wait_clock.add_sem_waits(nop.ins, bass_rust.ScopedClock({None: vc}))
    nc.sync.drain()
    nc.all_engine_barrier()
    assert self.sems is not None
    popped = nc._tile_sem_poison_stack.pop()
    assert popped is self._sem_poison
    nc.clear_and_free_semaphores(list(self.sems.allocated().values()))
    nc.all_engine_barrier()


_orig_lower = _tile.TileContext._lower_ordered_insts


def _patched_lower_ordered_insts(self, ordered):
    for bb_name, insts in ordered.items():
        new = []
        for inst in insts:
            si = getattr(inst, "sync_info", None)
            eng = getattr(inst, "engine", None)
            if (
                si is not None
                and si.on_wait
                and len(si.on_wait) > 1
                and eng is not None
                and isinstance(inst, mybir.Instruction)
            ):
                waits = list(si.on_wait)
                for w in waits[:-1]:
                    new.append(
                        mybir.InstNoOp(
                            name=self.nc.get_next_instruction_name(),
                            sync_info=mybir.SyncInfo(on_wait=[w], on_update=[]),
                            bass_nofuse=True,
                            engine=eng,
                        )
                    )
                inst.sync_info = mybir.SyncInfo(
                    on_wait=[waits[-1]], on_update=list(si.on_update)
                )
            new.append(inst)
        insts[:] = new
    return _orig_lower(self, ordered)


_tile.TileContext._drain_and_barrier = _patched_drain_and_barrier
_tile.TileContext._lower_ordered_insts = _patched_lower_ordered_insts

N, I, O, G = 16384, 128, 128, 16
NCORES = 8
NSH = N // NCORES
JT = 512
NJ = NSH // JT
F32 = mybir.dt.float32
F16 = mybir.dt.float16
A = mybir.AluOpType
AF = mybir.ActivationFunctionType
TWO_PI = 2.0 * np.pi
RC = 12582912.0
EPS = 1e-5
N_WARM = 8

# ---------------------------------------------------------------------------
# spec v7 expansions (for the host LS solve)
# ---------------------------------------------------------------------------


def _emul(e1, e2):
    out = {}

    def add(kind, g, v):
        if g < 0:
            g = -g
            if kind == "s":
                v = -v
        if g == 0:
            if kind == "s":
                return
            kind = "1"
        k = (kind, g)
        out[k] = out.get(k, 0.0) + v

    for (k1, g1), v1 in e1.items():
        for (k2, g2), v2 in e2.items():
            v = v1 * v2
            if k1 == "1" and k2 == "1":
                add("1", 0, v)
            elif k1 == "1":
                add(k2, g2, v)
            elif k2 == "1":
                add(k1, g1, v)
            elif k1 == "c" and k2 == "c":
                add("c", g1 + g2, 0.5 * v)
                add("c", g1 - g2, 0.5 * v)
            elif k1 == "s" and k2 == "s":
                add("c", g1 - g2, 0.5 * v)
                add("c", g1 + g2, -0.5 * v)
            elif k1 == "s" and k2 == "c":
                add("s", g1 + g2, 0.5 * v)
                add("s", g1 - g2, 0.5 * v)
            else:
                add("s", g1 + g2, 0.5 * v)
                add("s", g1 - g2, -0.5 * v)
    return {k: v for k, v in out.items() if abs(v) > 1e-15}


def _eaff(e, a, b):
    out = {k: a * v for k, v in e.items()}
    out[("1", 0)] = out.get(("1", 0), 0.0) + b
    return {k: v for k, v in out.items() if abs(v) > 1e-15}


PAIRS = [
    ("p3c", "sq2", "q1"), ("p3s", "t2", "q1"),
    ("p5c", "c4", "q1"), ("p5s", "s4", "q1"),
    ("p6c", "c4", "sq2"), ("p6s", "s4", "sq2"),
    ("p7c", "c4", "p3c"), ("p7s", "s4", "p3c"),
    ("p9c", "c8", "q1"), ("p9s", "c8", "s1"),
    ("p10c", "c8", "sq2"), ("p10s", "c8", "t2"),
    ("p11c", "c8", "p3c"), ("p11s", "c8", "p3s"),
    ("p12c", "c8", "c4"), ("p12s", "t8", "c4"),
    ("p13c", "p12c", "q1"), ("p13s", "p12s", "q1"),
    ("p14c", "p12c", "sq2"), ("p14s", "p12s", "sq2"),
    ("p15c", "p12c", "p3c"), ("p15s", "p12s", "p3c"),
]


def build_expansions():
    E = {"one": {("1", 0): 1.0}}
    E["s1"] = {("s", 1): 1.0}
    E["q1"] = {("1", 0): 0.5, ("c", 1): -0.5}
    E["s4"] = {("s", 4): 1.0}
    E["q4"] = {("1", 0): 0.5, ("c", 4): -0.5}
    E["c4"] = {("c", 4): 1.0}
    E["sq2"] = _emul(E["s1"], E["s1"])   # sin^2(x) = (1-c2)/2
    E["t2"] = _emul(E["q1"], E["s1"])
    E["sq8"] = _emul(E["s4"], E["s4"])   # sin^2(4x) = (1-c8)/2
    E["c8"] = _eaff(E["sq8"], -2.0, 1.0)
    E["t8"] = _emul(E["c4"], E["s4"])
    E["sq16"] = _emul(E["c8"], E["c8"])
    E["t16"] = _emul(E["c8"], E["t8"])
    for dst, a, b in PAIRS:
        E[dst] = _emul(E[a], E[b])
    return E


# GEMM accumulation order = tile production order
SPEC_BASIS = [
    "one", "s1", "q1", "s4", "q4", "sq2", "t2", "p3c", "p3s",
    "p5c", "p5s", "p6c", "p6s", "p7c", "p7s", "sq8", "t8",
    "p9c", "p9s", "p10c", "p10s", "p11c", "p11s", "p12c", "p12s",
    "sq16", "t16", "p13c", "p13s", "p14c", "p14s", "p15c", "p15s",
]
B = len(SPEC_BASIS)
assert B == 33

HARMONICS = [("1", 0)] + [("c", g) for g in range(1, G + 1)] + [
    ("s", g) for g in range(1, G + 1)
]


def solve_weights(cos_amp, sin_amp, bias):
    E = build_expansions()
    hidx = {h: k for k, h in enumerate(HARMONICS)}
    M = np.zeros((B, len(HARMONICS)))
    for bi, name in enumerate(SPEC_BASIS):
        for h, v in E[name].items():
            M[bi, hidx[h]] = v
    T = np.zeros((len(HARMONICS), I, O))
    T[0] = bias[None, :] / I
    for g in range(1, G + 1):
        T[hidx[("c", g)]] = cos_amp[:, :, g - 1].T
        T[hidx[("s", g)]] = sin_amp[:, :, g - 1].T
    piv = np.linalg.pinv(M.T)
    resid = np.abs(M.T @ piv - np.eye(len(HARMONICS))).max()
    assert resid < 1e-9, f"basis does not span harmonics: resid={resid}"
    W = np.einsum("bh,hio->bio", piv, T)
    W = W - W.mean(axis=2, keepdims=True)
    return W


# ---------------------------------------------------------------------------
# Device program
# ---------------------------------------------------------------------------

_NAMED = {"s1", "sh1", "q1", "s4", "sh4", "sq2", "t2", "c4", "c8", "t8",
          "p3c", "p3s", "p12c", "p12s"}


def build_device_program(beta_nonzero):
    nc = bass.Bass()
    x_in = nc.declare_dram_parameter("x_sh", [I, NSH], F32, isOutput=False)
    w_in = nc.declare_dram_parameter("w_all", [I, B * O], F16, isOutput=False)
    out_d = nc.declare_dram_parameter("out_sh", [O, NSH], F32, isOutput=True)

    with ExitStack() as ctx:
        tc = ctx.enter_context(TileContext(nc))
        cpool = ctx.enter_context(tc.tile_pool(name="const", bufs=1))
        xpool = ctx.enter_context(tc.tile_pool(name="xp", bufs=1))
        npool = ctx.enter_context(tc.tile_pool(name="named", bufs=1))
        rpool = ctx.enter_context(tc.tile_pool(name="ring", bufs=10))
        r5pool = ctx.enter_context(tc.tile_pool(name="ring5", bufs=16))
        pj = ctx.enter_context(tc.tile_pool(name="py", bufs=4, space="PSUM"))
        pw = ctx.enter_context(tc.tile_pool(name="pwarm", bufs=1, space="PSUM"))

        x = xpool.tile([I, NSH], F32, tag="x", name="x")
        wts = xpool.tile([I, B * O], F16, tag="wts", name="wts")
        H = NSH // 2
        nc.sync.dma_start(out=x[:, 0:H], in_=x_in[:, 0:H])
        nc.sync.dma_start(out=x[:, H:], in_=x_in[:, H:])
        WH = (B // 2) * O
        nc.sync.dma_start(out=wts[:, 0:WH], in_=w_in[:, 0:WH])
        nc.sync.dma_start(out=wts[:, WH:], in_=w_in[:, WH:])

        ones_col = cpool.tile([I, 1], F16, tag="ones_col", name="ones_col")
        nc.vector.memset(ones_col[:], 1.0)
        ones_mat = cpool.tile([I, O], F16, tag="ones_mat", name="ones_mat")
        nc.vector.memset(ones_mat[:], 1.0)
        ones_bc = ones_col[:].to_broadcast((I, JT))

        # ---- PE warmup ----
        warm = pw.tile([O, JT], F32, tag="warm", name="warm")
        for _ in range(N_WARM):
            nc.tensor.matmul(warm[:], ones_mat[:], ones_bc, start=True, stop=True)

        tiles = {}

        def tile16(name):
            if name in tiles:
                return tiles[name]
            pool = npool if name in _NAMED else rpool
            t = pool.tile([I, NSH], F16, tag=(name if name in _NAMED else "pr"),
                          name=name)
            tiles[name] = t
            return t

        ys = {}
        mm_count = [0] * NJ
        for j in range(NJ):
            ys[j] = pj.tile([O, JT], F32, tag="y", name=f"y{j}")

        s1c = float(np.float32(1.0 / TWO_PI))
        u1 = xpool.tile([I, NSH], F32, tag="u1", name="u1")
        t1 = xpool.tile([I, NSH], F32, tag="t1", name="t1")
        u4 = xpool.tile([I, NSH], F32, tag="u4", name="u4")
        t4f = xpool.tile([I, NSH], F32, tag="t4f", name="t4f")

        NARROW = ["p14c", "p14s", "p15c", "p15s"]
        nfac = {"p14c": ("p12c", "sq2"), "p14s": ("p12s", "sq2"),
                "p15c": ("p12c", "p3c"), "p15s": ("p12s", "p3c")}

        # fracs for BOTH halves up front: DVE chews half-b range reduction
        # while ACT builds half-a's carrier chain (fills the early DVE stall)
        for h in range(2):
            sl = slice(h * H, (h + 1) * H)
            nc.vector.tensor_scalar(u1[:, sl], x[:, sl], s1c, RC, A.mult, A.add)
            nc.vector.tensor_scalar(u1[:, sl], u1[:, sl], RC, None, A.subtract)
            nc.vector.scalar_tensor_tensor(
                t1[:, sl], x[:, sl], s1c, u1[:, sl], A.mult, A.subtract
            )
            nc.vector.tensor_scalar(u4[:, sl], t1[:, sl], 4.0, RC, A.mult, A.add)
            nc.vector.tensor_scalar(u4[:, sl], u4[:, sl], RC, None, A.subtract)
            nc.vector.scalar_tensor_tensor(
                t4f[:, sl], t1[:, sl], 4.0, u4[:, sl], A.mult, A.subtract
            )

        # entire op DAG per 1024-col half; each half feeds its two PSUM banks
        for h in range(2):
            sl = slice(h * H, (h + 1) * H)
            js = (2 * h, 2 * h + 1)

            def emit_mms(name):
                bi = SPEC_BASIS.index(name)
                for j in js:
                    if name == "one":
                        rhs = ones_bc
                    else:
                        rhs = tiles[name][:, j * JT : (j + 1) * JT]
                    nc.tensor.matmul(
                        ys[j][:],
                        wts[:, bi * O : (bi + 1) * O],
                        rhs,
                        start=(bi == 0),
                        stop=False,
                    )
                    mm_count[j] += 1

            def act(name, src, func, scale=1.0, bias=0.0):
                t = tile16(name)
                nc.scalar.activation(t[:, sl], src[:, sl], func, scale=scale,
                                     bias=bias)
                if name in SPEC_BASIS:
                    emit_mms(name)

            def mul(dst, a, b):
                t = tile16(dst)
                nc.vector.tensor_tensor(t[:, sl], tiles[a][:, sl],
                                        tiles[b][:, sl], A.mult)
                if dst in SPEC_BASIS:
                    emit_mms(dst)

            emit_mms("one")
            act("s1", t1, AF.Sin, scale=TWO_PI)
            act("sh1", t1, AF.Sin, scale=float(np.pi))
            act("s4", t4f, AF.Sin, scale=TWO_PI)
            act("sh4", t4f, AF.Sin, scale=float(np.pi))
            act("q4", tiles["sh4"], AF.Square)
            act("sq8", tiles["s4"], AF.Square)
            act("c4", tiles["q4"], AF.Copy, scale=-2.0, bias=1.0)
            act("c8", tiles["sq8"], AF.Copy, scale=-2.0, bias=1.0)
            act("sq16", tiles["c8"], AF.Square)
            mul("sq2", "s1", "s1")
            mul("q1", "sh1", "sh1")
            mul("t2", "q1", "s1")
            mul("p3c", "sq2", "q1")
            mul("p3s", "t2", "q1")
            mul("p5s", "s4", "q1")
            mul("p6s", "s4", "sq2")
            mul("p5c", "c4", "q1")
            mul("p6c", "c4", "sq2")
            mul("p7c", "c4", "p3c")
            mul("p7s", "s4", "p3c")
            mul("t8", "c4", "s4")
            mul("p9c", "c8", "q1")
            mul("p9s", "c8", "s1")
            mul("p10c", "c8", "sq2")
            mul("p10s", "c8", "t2")
            mul("p11c", "c8", "p3c")
            mul("p11s", "c8", "p3s")
            mul("p12c", "c8", "c4")
            mul("p12s", "t8", "c4")
            mul("t16", "c8", "t8")
            mul("p13c", "p12c", "q1")
            mul("p13s", "p12s", "q1")
            # last 4 tiles narrow per block: staggers y completion
            for j in js:
                slj = slice(j * JT, (j + 1) * JT)
                for name in NARROW:
                    a_, b_ = nfac[name]
                    nt = r5pool.tile([I, JT], F16, tag="nr", name=f"{name}_{j}")
                    nc.vector.tensor_tensor(nt[:], tiles[a_][:, slj],
                                            tiles[b_][:, slj], A.mult)
                    bi = SPEC_BASIS.index(name)
                    nc.tensor.matmul(
                        ys[j][:], wts[:, bi * O : (bi + 1) * O], nt[:],
                        start=False, stop=(bi == B - 1),
                    )
                    mm_count[j] += 1
                yo = r5pool.tile([O, JT], F32, tag="yo", name=f"yo{j}")
                nc.scalar.activation(yo[:], ys[j][:], AF.Copy)
                nc.gpsimd.dma_start(out=out_d[:, slj], in_=yo[:])
        assert all(c == B for c in mm_count), mm_count
    return nc


_NC_CACHE = {}


def kernel(x, cos_amplitudes, sin_amplitudes, bias, ln_gamma, ln_beta):
    from concourse.bass_utils import run_bass_kernel_spmd

    x = np.asarray(x, dtype=np.float32)
    ca = np.asarray(cos_amplitudes, dtype=np.float64)
    sa = np.asarray(sin_amplitudes, dtype=np.float64)
    bv = np.asarray(bias, dtype=np.float64)
    gv = np.asarray(ln_gamma, dtype=np.float32)
    be = np.asarray(ln_beta, dtype=np.float32)

    W = solve_weights(ca, sa, bv)
    w_all = np.ascontiguousarray(
        W.transpose(1, 0, 2).reshape(I, B * O)
    ).astype(np.float16)

    xT = np.ascontiguousarray(x.T)

    if 0 not in _NC_CACHE:
        _NC_CACHE[0] = build_device_program(False)
    nc = _NC_CACHE[0]

    in_maps = []
    for c in range(NCORES):
        in_maps.append(
            {
                "x_sh": np.ascontiguousarray(xT[:, c * NSH : (c + 1) * NSH]),
                "w_all": w_all,
            }
        )
    res = run_bass_kernel_spmd(nc, in_maps, list(range(NCORES)))
    outs = [res.results[c]["out_sh"] for c in range(NCORES)]
    full = np.concatenate(outs, axis=1)  # [O, N], centered over O
    y = np.ascontiguousarray(full.T).astype(np.float32)  # [N, O]
    var = np.mean(y * y, axis=1, keepdims=True)
    out = y * (1.0 / np.sqrt(var + EPS)) * gv[None, :] + be[None, :]
    return out.astype(np.float32)


# revision 25
# speedup vs baseline: 1.0381x; 1.0381x over previous
"""FourierKAN layer (N=16384, I=128, O=128, G=16) on 8 Trainium2 NeuronCores.

Design (data-parallel over N, 2048 rows/core):
 - Basis of 33 fp16 tiles spanning harmonics {1, cos gx, sin gx, g<=16}:
   directs {1,4} via fp32 round-constant range reduction + ACT Sin;
   pure cosine carriers c4, c8 via Square+affine-Copy; sin-side chain and
   20 two-factor products (affine q-carriers; host LS absorbs the mixing).
 - Host solves exact LS weights W[b,i,o] (fp64) mapping basis -> amplitudes,
   centered over o (folds LayerNorm mean-subtraction into the weights).
 - Op DAG runs per 1024-col half (balances DVE fixed overhead against
   pipeline latency); each produced tile immediately feeds its two 512-col
   PSUM banks' accumulating matmuls. The last 4 tiles are produced per
   512-block to stagger bank completion and overlap the output DMAs.
 - Pool/GpSimd does no elementwise compute (its tensor ops run ~3.3x slower
   AND steal DVE SBUF ports); PE warmup matmuls counter the HAM clock gate.
 - Device returns centered pre-LN y [O, N]; host applies the cheap
   normalization y*rsqrt(mean(y^2)+eps)*gamma+beta and the transpose.
"""
import sys

sys.path.insert(0, "/opt/trn_rl_repo")

import numpy as np

import concourse.bass as bass
import concourse.mybir as mybir
from concourse.tile import TileContext
from contextlib import ExitStack

import bass_rust
from concourse import tile as _tile


def _patched_drain_and_barrier(self, tick_clock, wait_clock):
    nc = self.nc
    gc = tick_clock.global_clock
    n = len(gc)
    for p in range(n):
        if gc[p] > 0:
            vc = bass_rust.VectorClock([0] * n)
            vc.require_at_least(p, gc[p])
            nop = nc.sync.nop(hint="drain_wait_carrier", nofuse=True)
            wait_clock.add_sem_waits(nop.ins, bass_rust.ScopedClock({None: vc}))
    nc.sync.drain()
    nc.all_engine_barrier()
    assert self.sems is not None
    popped = nc._tile_sem_poison_stack.pop()
    assert popped is self._sem_poison
    nc.clear_and_free_semaphores(list(self.sems.allocated().values()))
    nc.all_engine_barrier()


_orig_lower = _tile.TileContext._lower_ordered_insts


def _patched_lower_ordered_insts(self, ordered):
    for bb_name, insts in ordered.items():
        new = []
        for inst in insts:
            si = getattr(inst, "sync_info", None)
            eng = getattr(inst, "engine", None)
            if (
                si is not None
                and si.on_wait
                and len(si.on_wait) > 1
                and eng is not None
                and isinstance(inst, mybir.Instruction)
            ):
                waits = list(si.on_wait)
                for w in waits[:-1]:
                    new.append(
                        mybir.InstNoOp(
                            name=self.nc.get_next_instruction_name(),
                            sync_info=mybir.SyncInfo(on_wait=[w], on_update=[]),
                            bass_nofuse=True,
                            engine=eng,
                        )
                    )
                inst.sync_info = mybir.SyncInfo(
                    on_wait=[waits[-1]], on_update=list(si.on_update)
                )
            new.append(inst)
        insts[:] = new
    return _orig_lower(self, ordered)


_tile.TileContext._drain_and_barrier = _patched_drain_and_barrier
_tile.TileContext._lower_ordered_insts = _patched_lower_ordered_insts

N, I, O, G = 16384, 128, 128, 16
NCORES = 8
NSH = N // NCORES
JT = 512
NJ = NSH // JT
F32 = mybir.dt.float32
F16 = mybir.dt.float16
A = mybir.AluOpType
AF = mybir.ActivationFunctionType
TWO_PI = 2.0 * np.pi
RC = 12582912.0
EPS = 1e-5
N_WARM = 8

# ---------------------------------------------------------------------------
# spec v7 expansions (for the host LS solve)
# ---------------------------------------------------------------------------


def _emul(e1, e2):
    out = {}

    def add(kind, g, v):
        if g < 0:
            g = -g
            if kind == "s":
                v = -v
        if g == 0:
            if kind == "s":
                return
            kind = "1"
        k = (kind, g)
        out[k] = out.get(k, 0.0) + v

    for (k1, g1), v1 in e1.items():
        for (k2, g2), v2 in e2.items():
            v = v1 * v2
            if k1 == "1" and k2 == "1":
                add("1", 0, v)
            elif k1 == "1":
                add(k2, g2, v)
            elif k2 == "1":
                add(k1, g1, v)
            elif k1 == "c" and k2 == "c":
                add("c", g1 + g2, 0.5 * v)
                add("c", g1 - g2, 0.5 * v)
            elif k1 == "s" and k2 == "s":
                add("c", g1 - g2, 0.5 * v)
                add("c", g1 + g2, -0.5 * v)
            elif k1 == "s" and k2 == "c":
                add("s", g1 + g2, 0.5 * v)
                add("s", g1 - g2, 0.5 * v)
            else:
                add("s", g1 + g2, 0.5 * v)
                add("s", g1 - g2, -0.5 * v)
    return {k: v for k, v in out.items() if abs(v) > 1e-15}


def _eaff(e, a, b):
    out = {k: a * v for k, v in e.items()}
    out[("1", 0)] = out.get(("1", 0), 0.0) + b
    return {k: v for k, v in out.items() if abs(v) > 1e-15}


PAIRS = [
    ("p3c", "sq2", "q1"), ("p3s", "t2", "q1"),
    ("p5c", "c4", "q1"), ("p5s", "s4", "q1"),
    ("p6c", "c4", "sq2"), ("p6s", "s4", "sq2"),
    ("p7c", "c4", "p3c"), ("p7s", "s4", "p3c"),
    ("p9c", "c8", "q1"), ("p9s", "c8", "s1"),
    ("p10c", "c8", "sq2"), ("p10s", "c8", "t2"),
    ("p11c", "c8", "p3c"), ("p11s", "c8", "p3s"),
    ("p12c", "c8", "c4"), ("p12s", "t8", "c4"),
    ("p13c", "p12c", "q1"), ("p13s", "p12s", "q1"),
    ("p14c", "p12c", "sq2"), ("p14s", "p12s", "sq2"),
    ("p15c", "p12c", "p3c"), ("p15s", "p12s", "p3c"),
]


def build_expansions():
    E = {"one": {("1", 0): 1.0}}
    E["s1"] = {("s", 1): 1.0}
    E["q1"] = {("1", 0): 0.5, ("c", 1): -0.5}
    E["s4"] = {("s", 4): 1.0}
    E["q4"] = {("1", 0): 0.5, ("c", 4): -0.5}
    E["c4"] = {("c", 4): 1.0}
    E["sq2"] = _emul(E["s1"], E["s1"])   # sin^2(x) = (1-c2)/2
    E["t2"] = _emul(E["q1"], E["s1"])
    E["sq8"] = _emul(E["s4"], E["s4"])   # sin^2(4x) = (1-c8)/2
    E["c8"] = _eaff(E["sq8"], -2.0, 1.0)
    E["t8"] = _emul(E["c4"], E["s4"])
    E["sq16"] = _emul(E["c8"], E["c8"])
    E["t16"] = _emul(E["c8"], E["t8"])
    for dst, a, b in PAIRS:
        E[dst] = _emul(E[a], E[b])
    return E


# GEMM accumulation order = tile production order
SPEC_BASIS = [
    "one", "s1", "q1", "s4", "q4", "sq2", "t2", "p3c", "p3s",
    "p5c", "p5s", "p6c", "p6s", "p7c", "p7s", "sq8", "t8",
    "p9c", "p9s", "p10c", "p10s", "p11c", "p11s", "p12c", "p12s",
    "sq16", "t16", "p13c", "p13s", "p14c", "p14s", "p15c", "p15s",
]
B = len(SPEC_BASIS)
assert B == 33

HARMONICS = [("1", 0)] + [("c", g) for g in range(1, G + 1)] + [
    ("s", g) for g in range(1, G + 1)
]


def solve_weights(cos_amp, sin_amp, bias):
    E = build_expansions()
    hidx = {h: k for k, h in enumerate(HARMONICS)}
    M = np.zeros((B, len(HARMONICS)))
    for bi, name in enumerate(SPEC_BASIS):
        for h, v in E[name].items():
            M[bi, hidx[h]] = v
    T = np.zeros((len(HARMONICS), I, O))
    T[0] = bias[None, :] / I
    for g in range(1, G + 1):
        T[hidx[("c", g)]] = cos_amp[:, :, g - 1].T
        T[hidx[("s", g)]] = sin_amp[:, :, g - 1].T
    piv = np.linalg.pinv(M.T)
    resid = np.abs(M.T @ piv - np.eye(len(HARMONICS))).max()
    assert resid < 1e-9, f"basis does not span harmonics: resid={resid}"
    W = np.einsum("bh,hio->bio", piv, T)
    W = W - W.mean(axis=2, keepdims=True)
    return W


# ---------------------------------------------------------------------------
# Device program
# ---------------------------------------------------------------------------

_NAMED = {"s1", "sh1", "q1", "s4", "sh4", "sq2", "t2", "c4", "c8", "t8",
          "p3c", "p3s", "p12c", "p12s"}


def build_device_program(beta_nonzero):
    nc = bass.Bass()
    x_in = nc.declare_dram_parameter("x_sh", [I, NSH], F32, isOutput=False)
    w_in = nc.declare_dram_parameter("w_all", [I, B * O], F16, isOutput=False)
    out_d = nc.declare_dram_parameter("out_sh", [O, NSH], F32, isOutput=True)

    with ExitStack() as ctx:
        tc = ctx.enter_context(TileContext(nc))
        cpool = ctx.enter_context(tc.tile_pool(name="const", bufs=1))
        xpool = ctx.enter_context(tc.tile_pool(name="xp", bufs=1))
        npool = ctx.enter_context(tc.tile_pool(name="named", bufs=1))
        rpool = ctx.enter_context(tc.tile_pool(name="ring", bufs=10))
        r5pool = ctx.enter_context(tc.tile_pool(name="ring5", bufs=16))
        pj = ctx.enter_context(tc.tile_pool(name="py", bufs=4, space="PSUM"))
        pw = ctx.enter_context(tc.tile_pool(name="pwarm", bufs=1, space="PSUM"))

        x = xpool.tile([I, NSH], F32, tag="x", name="x")
        wts = xpool.tile([I, B * O], F16, tag="wts", name="wts")
        H = NSH // 2
        Q = H // 2
        nc.sync.dma_start(out=x[:, 0:Q], in_=x_in[:, 0:Q])
        nc.sync.dma_start(out=x[:, Q:H], in_=x_in[:, Q:H])
        nc.sync.dma_start(out=x[:, H:], in_=x_in[:, H:])
        WH = (B // 2) * O
        nc.sync.dma_start(out=wts[:, 0:WH], in_=w_in[:, 0:WH])
        nc.sync.dma_start(out=wts[:, WH:], in_=w_in[:, WH:])

        ones_col = cpool.tile([I, 1], F16, tag="ones_col", name="ones_col")
        nc.vector.memset(ones_col[:], 1.0)
        ones_mat = cpool.tile([I, O], F16, tag="ones_mat", name="ones_mat")
        nc.vector.memset(ones_mat[:], 1.0)
        ones_bc = ones_col[:].to_broadcast((I, JT))

        # ---- PE warmup ----
        warm = pw.tile([O, JT], F32, tag="warm", name="warm")
        for _ in range(N_WARM):
            nc.tensor.matmul(warm[:], ones_mat[:], ones_bc, start=True, stop=True)

        tiles = {}

        def tile16(name):
            if name in tiles:
                return tiles[name]
            pool = npool if name in _NAMED else rpool
            t = pool.tile([I, NSH], F16, tag=(name if name in _NAMED else "pr"),
                          name=name)
            tiles[name] = t
            return t

        ys = {}
        mm_count = [0] * NJ
        for j in range(NJ):
            ys[j] = pj.tile([O, JT], F32, tag="y", name=f"y{j}")

        s1c = float(np.float32(1.0 / TWO_PI))
        u1 = xpool.tile([I, NSH], F32, tag="u1", name="u1")
        t1 = xpool.tile([I, NSH], F32, tag="t1", name="t1")
        u4 = xpool.tile([I, NSH], F32, tag="u4", name="u4")
        t4f = xpool.tile([I, NSH], F32, tag="t4f", name="t4f")

        NARROW = ["p14c", "p14s", "p15c", "p15s"]
        nfac = {"p14c": ("p12c", "sq2"), "p14s": ("p12s", "sq2"),
                "p15c": ("p12c", "p3c"), "p15s": ("p12s", "p3c")}

        # fracs for BOTH halves up front: DVE chews half-b range reduction
        # while ACT builds half-a's carrier chain (fills the early DVE stall).
        # half-a runs as two quarters so DVE starts on the first-arrived chunk
        for sl in (slice(0, Q), slice(Q, H), slice(H, NSH)):
            nc.vector.tensor_scalar(u1[:, sl], x[:, sl], s1c, RC, A.mult, A.add)
            nc.vector.tensor_scalar(u1[:, sl], u1[:, sl], RC, None, A.subtract)
            nc.vector.scalar_tensor_tensor(
                t1[:, sl], x[:, sl], s1c, u1[:, sl], A.mult, A.subtract
            )
            nc.vector.tensor_scalar(u4[:, sl], t1[:, sl], 4.0, RC, A.mult, A.add)
            nc.vector.tensor_scalar(u4[:, sl], u4[:, sl], RC, None, A.subtract)
            nc.vector.scalar_tensor_tensor(
                t4f[:, sl], t1[:, sl], 4.0, u4[:, sl], A.mult, A.subtract
            )

        # entire op DAG per 1024-col half; each half feeds its two PSUM banks
        for h in range(2):
            sl = slice(h * H, (h + 1) * H)
            js = (2 * h, 2 * h + 1)

            def emit_mms(name):
                bi = SPEC_BASIS.index(name)
                for j in js:
                    if name == "one":
                        rhs = ones_bc
                    else:
                        rhs = tiles[name][:, j * JT : (j + 1) * JT]
                    nc.tensor.matmul(
                        ys[j][:],
                        wts[:, bi * O : (bi + 1) * O],
                        rhs,
                        start=(bi == 0),
                        stop=False,
                    )
                    mm_count[j] += 1

            def act(name, src, func, scale=1.0, bias=0.0):
                t = tile16(name)
                nc.scalar.activation(t[:, sl], src[:, sl], func, scale=scale,
                                     bias=bias)
                if name in SPEC_BASIS:
                    emit_mms(name)

            def mul(dst, a, b):
                t = tile16(dst)
                nc.vector.tensor_tensor(t[:, sl], tiles[a][:, sl],
                                        tiles[b][:, sl], A.mult)
                if dst in SPEC_BASIS:
                    emit_mms(dst)

            emit_mms("one")
            act("s1", t1, AF.Sin, scale=TWO_PI)
            act("sh1", t1, AF.Sin, scale=float(np.pi))
            act("s4", t4f, AF.Sin, scale=TWO_PI)
            act("sh4", t4f, AF.Sin, scale=float(np.pi))
            act("q4", tiles["sh4"], AF.Square)
            act("sq8", tiles["s4"], AF.Square)
            act("c4", tiles["q4"], AF.Copy, scale=-2.0, bias=1.0)
            act("c8", tiles["sq8"], AF.Copy, scale=-2.0, bias=1.0)
            act("sq16", tiles["c8"], AF.Square)
            mul("sq2", "s1", "s1")
            mul("q1", "sh1", "sh1")
            mul("t2", "q1", "s1")
            mul("p3c", "sq2", "q1")
            mul("p3s", "t2", "q1")
            mul("p5s", "s4", "q1")
            mul("p6s", "s4", "sq2")
            mul("p5c", "c4", "q1")
            mul("p6c", "c4", "sq2")
            mul("p7c", "c4", "p3c")
            mul("p7s", "s4", "p3c")
            mul("t8", "c4", "s4")
            mul("p9c", "c8", "q1")
            mul("p9s", "c8", "s1")
            mul("p10c", "c8", "sq2")
            mul("p10s", "c8", "t2")
            mul("p11c", "c8", "p3c")
            mul("p11s", "c8", "p3s")
            mul("p12c", "c8", "c4")
            mul("p12s", "t8", "c4")
            mul("t16", "c8", "t8")
            mul("p13c", "p12c", "q1")
            mul("p13s", "p12s", "q1")
            # last 4 tiles narrow per block: staggers y completion
            for j in js:
                slj = slice(j * JT, (j + 1) * JT)
                for name in NARROW:
                    a_, b_ = nfac[name]
                    nt = r5pool.tile([I, JT], F16, tag="nr", name=f"{name}_{j}")
                    nc.vector.tensor_tensor(nt[:], tiles[a_][:, slj],
                                            tiles[b_][:, slj], A.mult)
                    bi = SPEC_BASIS.index(name)
                    nc.tensor.matmul(
                        ys[j][:], wts[:, bi * O : (bi + 1) * O], nt[:],
                        start=False, stop=(bi == B - 1),
                    )
                    mm_count[j] += 1
                yo = r5pool.tile([O, JT], F32, tag="yo", name=f"yo{j}")
                nc.scalar.activation(yo[:], ys[j][:], AF.Copy)
                nc.gpsimd.dma_start(out=out_d[:, slj], in_=yo[:])
        assert all(c == B for c in mm_count), mm_count
    return nc


_NC_CACHE = {}


def kernel(x, cos_amplitudes, sin_amplitudes, bias, ln_gamma, ln_beta):
    from concourse.bass_utils import run_bass_kernel_spmd

    x = np.asarray(x, dtype=np.float32)
    ca = np.asarray(cos_amplitudes, dtype=np.float64)
    sa = np.asarray(sin_amplitudes, dtype=np.float64)
    bv = np.asarray(bias, dtype=np.float64)
    gv = np.asarray(ln_gamma, dtype=np.float32)
    be = np.asarray(ln_beta, dtype=np.float32)

    W = solve_weights(ca, sa, bv)
    w_all = np.ascontiguousarray(
        W.transpose(1, 0, 2).reshape(I, B * O)
    ).astype(np.float16)

    xT = np.ascontiguousarray(x.T)

    if 0 not in _NC_CACHE:
        _NC_CACHE[0] = build_device_program(False)
    nc = _NC_CACHE[0]

    in_maps = []
    for c in range(NCORES):
        in_maps.append(
            {
                "x_sh": np.ascontiguousarray(xT[:, c * NSH : (c + 1) * NSH]),
                "w_all": w_all,
            }
        )
    res = run_bass_kernel_spmd(nc, in_maps, list(range(NCORES)))
    outs = [res.results[c]["out_sh"] for c in range(NCORES)]
    full = np.concatenate(outs, axis=1)  # [O, N], centered over O
    y = np.ascontiguousarray(full.T).astype(np.float32)  # [N, O]
    var = np.mean(y * y, axis=1, keepdims=True)
    out = y * (1.0 / np.sqrt(var + EPS)) * gv[None, :] + be[None, :]
    return out.astype(np.float32)


# revision 26
# speedup vs baseline: 1.0410x; 1.0028x over previous
"""FourierKAN layer (N=16384, I=128, O=128, G=16) on 8 Trainium2 NeuronCores.

Design (data-parallel over N, 2048 rows/core):
 - Basis of 33 fp16 tiles spanning harmonics {1, cos gx, sin gx, g<=16}:
   directs {1,4} via fp32 round-constant range reduction + ACT Sin;
   pure cosine carriers c4, c8 via Square+affine-Copy; sin-side chain and
   20 two-factor products (affine q-carriers; host LS absorbs the mixing).
 - Host solves exact LS weights W[b,i,o] (fp64) mapping basis -> amplitudes,
   centered over o (folds LayerNorm mean-subtraction into the weights).
 - Op DAG runs per 1024-col half (balances DVE fixed overhead against
   pipeline latency); each produced tile immediately feeds its two 512-col
   PSUM banks' accumulating matmuls. The last 4 tiles are produced per
   512-block to stagger bank completion and overlap the output DMAs.
 - Pool/GpSimd does no elementwise compute (its tensor ops run ~3.3x slower
   AND steal DVE SBUF ports); PE warmup matmuls counter the HAM clock gate.
 - Device returns centered pre-LN y [O, N]; host applies the cheap
   normalization y*rsqrt(mean(y^2)+eps)*gamma+beta and the transpose.
"""
import sys

sys.path.insert(0, "/opt/trn_rl_repo")

import numpy as np

import concourse.bass as bass
import concourse.mybir as mybir
from concourse.tile import TileContext
from contextlib import ExitStack

import bass_rust
from concourse import tile as _tile


def _patched_drain_and_barrier(self, tick_clock, wait_clock):
    nc = self.nc
    gc = tick_clock.global_clock
    n = len(gc)
    for p in range(n):
        if gc[p] > 0:
            vc = bass_rust.VectorClock([0] * n)
            vc.require_at_least(p, gc[p])
            nop = nc.sync.nop(hint="drain_wait_carrier", nofuse=True)
            wait_clock.add_sem_waits(nop.ins, bass_rust.ScopedClock({None: vc}))
    nc.sync.drain()
    nc.all_engine_barrier()
    assert self.sems is not None
    popped = nc._tile_sem_poison_stack.pop()
    assert popped is self._sem_poison
    nc.clear_and_free_semaphores(list(self.sems.allocated().values()))
    nc.all_engine_barrier()


_orig_lower = _tile.TileContext._lower_ordered_insts


def _patched_lower_ordered_insts(self, ordered):
    for bb_name, insts in ordered.items():
        new = []
        for inst in insts:
            si = getattr(inst, "sync_info", None)
            eng = getattr(inst, "engine", None)
            if (
                si is not None
                and si.on_wait
                and len(si.on_wait) > 1
                and eng is not None
                and isinstance(inst, mybir.Instruction)
            ):
                waits = list(si.on_wait)
                for w in waits[:-1]:
                    new.append(
                        mybir.InstNoOp(
                            name=self.nc.get_next_instruction_name(),
                            sync_info=mybir.SyncInfo(on_wait=[w], on_update=[]),
                            bass_nofuse=True,
                            engine=eng,
                        )
                    )
                inst.sync_info = mybir.SyncInfo(
                    on_wait=[waits[-1]], on_update=list(si.on_update)
                )
            new.append(inst)
        insts[:] = new
    return _orig_lower(self, ordered)


_tile.TileContext._drain_and_barrier = _patched_drain_and_barrier
_tile.TileContext._lower_ordered_insts = _patched_lower_ordered_insts

N, I, O, G = 16384, 128, 128, 16
NCORES = 8
NSH = N // NCORES
JT = 512
NJ = NSH // JT
F32 = mybir.dt.float32
F16 = mybir.dt.float16
A = mybir.AluOpType
AF = mybir.ActivationFunctionType
TWO_PI = 2.0 * np.pi
RC = 12582912.0
EPS = 1e-5
N_WARM = 8

# ---------------------------------------------------------------------------
# spec v7 expansions (for the host LS solve)
# ---------------------------------------------------------------------------


def _emul(e1, e2):
    out = {}

    def add(kind, g, v):
        if g < 0:
            g = -g
            if kind == "s":
                v = -v
        if g == 0:
            if kind == "s":
                return
            kind = "1"
        k = (kind, g)
        out[k] = out.get(k, 0.0) + v

    for (k1, g1), v1 in e1.items():
        for (k2, g2), v2 in e2.items():
            v = v1 * v2
            if k1 == "1" and k2 == "1":
                add("1", 0, v)
            elif k1 == "1":
                add(k2, g2, v)
            elif k2 == "1":
                add(k1, g1, v)
            elif k1 == "c" and k2 == "c":
                add("c", g1 + g2, 0.5 * v)
                add("c", g1 - g2, 0.5 * v)
            elif k1 == "s" and k2 == "s":
                add("c", g1 - g2, 0.5 * v)
                add("c", g1 + g2, -0.5 * v)
            elif k1 == "s" and k2 == "c":
                add("s", g1 + g2, 0.5 * v)
                add("s", g1 - g2, 0.5 * v)
            else:
                add("s", g1 + g2, 0.5 * v)
                add("s", g1 - g2, -0.5 * v)
    return {k: v for k, v in out.items() if abs(v) > 1e-15}


def _eaff(e, a, b):
    out = {k: a * v for k, v in e.items()}
    out[("1", 0)] = out.get(("1", 0), 0.0) + b
    return {k: v for k, v in out.items() if abs(v) > 1e-15}


PAIRS = [
    ("p3c", "sq2", "q1"), ("p3s", "t2", "q1"),
    ("p5c", "c4", "q1"), ("p5s", "s4", "q1"),
    ("p6c", "c4", "sq2"), ("p6s", "s4", "sq2"),
    ("p7c", "c4", "p3c"), ("p7s", "s4", "p3c"),
    ("p9c", "c8", "q1"), ("p9s", "c8", "s1"),
    ("p10c", "c8", "sq2"), ("p10s", "c8", "t2"),
    ("p11c", "c8", "p3c"), ("p11s", "c8", "p3s"),
    ("p12c", "c8", "c4"), ("p12s", "t8", "c4"),
    ("p13c", "p12c", "q1"), ("p13s", "p12s", "q1"),
    ("p14c", "p12c", "sq2"), ("p14s", "p12s", "sq2"),
    ("p15c", "p12c", "p3c"), ("p15s", "p12s", "p3c"),
]


def build_expansions():
    E = {"one": {("1", 0): 1.0}}
    E["s1"] = {("s", 1): 1.0}
    E["q1"] = {("1", 0): 0.5, ("c", 1): -0.5}
    E["s4"] = {("s", 4): 1.0}
    E["q4"] = {("1", 0): 0.5, ("c", 4): -0.5}
    E["c4"] = {("c", 4): 1.0}
    E["sq2"] = _emul(E["s1"], E["s1"])   # sin^2(x) = (1-c2)/2
    E["t2"] = _emul(E["q1"], E["s1"])
    E["sq8"] = _emul(E["s4"], E["s4"])   # sin^2(4x) = (1-c8)/2
    E["c8"] = _eaff(E["sq8"], -2.0, 1.0)
    E["t8"] = _emul(E["c4"], E["s4"])
    E["sq16"] = _emul(E["c8"], E["c8"])
    E["t16"] = _emul(E["c8"], E["t8"])
    for dst, a, b in PAIRS:
        E[dst] = _emul(E[a], E[b])
    return E


# GEMM accumulation order = tile production order
SPEC_BASIS = [
    "one", "s1", "q1", "s4", "q4", "sq2", "t2", "p3c", "p3s",
    "p5c", "p5s", "p6c", "p6s", "p7c", "p7s", "sq8", "t8",
    "p9c", "p9s", "p10c", "p10s", "p11c", "p11s", "p12c", "p12s",
    "sq16", "t16", "p13c", "p13s", "p14c", "p14s", "p15c", "p15s",
]
B = len(SPEC_BASIS)
assert B == 33

HARMONICS = [("1", 0)] + [("c", g) for g in range(1, G + 1)] + [
    ("s", g) for g in range(1, G + 1)
]


def solve_weights(cos_amp, sin_amp, bias):
    E = build_expansions()
    hidx = {h: k for k, h in enumerate(HARMONICS)}
    M = np.zeros((B, len(HARMONICS)))
    for bi, name in enumerate(SPEC_BASIS):
        for h, v in E[name].items():
            M[bi, hidx[h]] = v
    T = np.zeros((len(HARMONICS), I, O))
    T[0] = bias[None, :] / I
    for g in range(1, G + 1):
        T[hidx[("c", g)]] = cos_amp[:, :, g - 1].T
        T[hidx[("s", g)]] = sin_amp[:, :, g - 1].T
    piv = np.linalg.pinv(M.T)
    resid = np.abs(M.T @ piv - np.eye(len(HARMONICS))).max()
    assert resid < 1e-9, f"basis does not span harmonics: resid={resid}"
    W = np.einsum("bh,hio->bio", piv, T)
    W = W - W.mean(axis=2, keepdims=True)
    return W


# ---------------------------------------------------------------------------
# Device program
# ---------------------------------------------------------------------------

_NAMED = {"s1", "sh1", "q1", "s4", "sh4", "sq2", "t2", "c4", "c8", "t8",
          "p3c", "p3s", "p12c", "p12s"}


def build_device_program(beta_nonzero):
    nc = bass.Bass()
    x_in = nc.declare_dram_parameter("x_sh", [I, NSH], F32, isOutput=False)
    w_in = nc.declare_dram_parameter("w_all", [I, B * O], F16, isOutput=False)
    out_d = nc.declare_dram_parameter("out_sh", [O, NSH], F32, isOutput=True)

    with ExitStack() as ctx:
        tc = ctx.enter_context(TileContext(nc))
        cpool = ctx.enter_context(tc.tile_pool(name="const", bufs=1))
        xpool = ctx.enter_context(tc.tile_pool(name="xp", bufs=1))
        npool = ctx.enter_context(tc.tile_pool(name="named", bufs=1))
        rpool = ctx.enter_context(tc.tile_pool(name="ring", bufs=10))
        r5pool = ctx.enter_context(tc.tile_pool(name="ring5", bufs=16))
        pj = ctx.enter_context(tc.tile_pool(name="py", bufs=4, space="PSUM"))
        pw = ctx.enter_context(tc.tile_pool(name="pwarm", bufs=1, space="PSUM"))

        x = xpool.tile([I, NSH], F32, tag="x", name="x")
        wts = xpool.tile([I, B * O], F16, tag="wts", name="wts")
        H = NSH // 2
        Q = H // 2
        nc.sync.dma_start(out=x[:, 0:Q], in_=x_in[:, 0:Q])
        nc.sync.dma_start(out=x[:, Q:H], in_=x_in[:, Q:H])
        nc.sync.dma_start(out=x[:, H:], in_=x_in[:, H:])
        WH = (B // 2) * O
        nc.sync.dma_start(out=wts[:, 0:WH], in_=w_in[:, 0:WH])
        nc.sync.dma_start(out=wts[:, WH:], in_=w_in[:, WH:])

        ones_col = cpool.tile([I, 1], F16, tag="ones_col", name="ones_col")
        nc.vector.memset(ones_col[:], 1.0)
        ones_mat = cpool.tile([I, O], F16, tag="ones_mat", name="ones_mat")
        nc.vector.memset(ones_mat[:], 1.0)
        ones_bc = ones_col[:].to_broadcast((I, JT))

        # ---- PE warmup ----
        warm = pw.tile([O, JT], F32, tag="warm", name="warm")
        for _ in range(N_WARM):
            nc.tensor.matmul(warm[:], ones_mat[:], ones_bc, start=True, stop=True)

        tiles = {}

        def tile16(name):
            if name in tiles:
                return tiles[name]
            pool = npool if name in _NAMED else rpool
            t = pool.tile([I, NSH], F16, tag=(name if name in _NAMED else "pr"),
                          name=name)
            tiles[name] = t
            return t

        ys = {}
        mm_count = [0] * NJ
        for j in range(NJ):
            ys[j] = pj.tile([O, JT], F32, tag="y", name=f"y{j}")

        s1c = float(np.float32(1.0 / TWO_PI))
        u1 = xpool.tile([I, NSH], F32, tag="u1", name="u1")
        t1 = xpool.tile([I, NSH], F32, tag="t1", name="t1")
        u4 = xpool.tile([I, NSH], F32, tag="u4", name="u4")
        t4f = xpool.tile([I, NSH], F32, tag="t4f", name="t4f")

        NARROW = ["p14c", "p14s", "p15c", "p15s"]
        nfac = {"p14c": ("p12c", "sq2"), "p14s": ("p12s", "sq2"),
                "p15c": ("p12c", "p3c"), "p15s": ("p12s", "p3c")}

        # fracs for BOTH halves up front: DVE chews half-b range reduction
        # while ACT builds half-a's carrier chain (fills the early DVE stall).
        # half-a runs as two quarters so DVE starts on the first-arrived chunk
        for sl in (slice(0, Q), slice(Q, H), slice(H, NSH)):
            nc.vector.tensor_scalar(u1[:, sl], x[:, sl], s1c, RC, A.mult, A.add)
            nc.vector.tensor_scalar(u1[:, sl], u1[:, sl], RC, None, A.subtract)
            nc.vector.scalar_tensor_tensor(
                t1[:, sl], x[:, sl], s1c, u1[:, sl], A.mult, A.subtract
            )
            nc.vector.tensor_scalar(u4[:, sl], t1[:, sl], 4.0, RC, A.mult, A.add)
            nc.vector.tensor_scalar(u4[:, sl], u4[:, sl], RC, None, A.subtract)
            nc.vector.scalar_tensor_tensor(
                t4f[:, sl], t1[:, sl], 4.0, u4[:, sl], A.mult, A.subtract
            )

        # entire op DAG per 1024-col half; each half feeds its two PSUM banks
        for h in range(2):
            sl = slice(h * H, (h + 1) * H)
            js = (2 * h, 2 * h + 1)

            def emit_mms(name):
                bi = SPEC_BASIS.index(name)
                for j in js:
                    if name == "one":
                        rhs = ones_bc
                    else:
                        rhs = tiles[name][:, j * JT : (j + 1) * JT]
                    nc.tensor.matmul(
                        ys[j][:],
                        wts[:, bi * O : (bi + 1) * O],
                        rhs,
                        start=(bi == 0),
                        stop=False,
                    )
                    mm_count[j] += 1

            def act(name, src, func, scale=1.0, bias=0.0):
                t = tile16(name)
                nc.scalar.activation(t[:, sl], src[:, sl], func, scale=scale,
                                     bias=bias)
                if name in SPEC_BASIS:
                    emit_mms(name)

            def mul(dst, a, b):
                t = tile16(dst)
                nc.vector.tensor_tensor(t[:, sl], tiles[a][:, sl],
                                        tiles[b][:, sl], A.mult)
                if dst in SPEC_BASIS:
                    emit_mms(dst)

            emit_mms("one")
            act("s1", t1, AF.Sin, scale=TWO_PI)
            act("sh1", t1, AF.Sin, scale=float(np.pi))
            act("s4", t4f, AF.Sin, scale=TWO_PI)
            act("sh4", t4f, AF.Sin, scale=float(np.pi))
            act("q4", tiles["sh4"], AF.Square)
            act("sq8", tiles["s4"], AF.Square)
            act("c4", tiles["q4"], AF.Copy, scale=-2.0, bias=1.0)
            act("c8", tiles["sq8"], AF.Copy, scale=-2.0, bias=1.0)
            act("sq16", tiles["c8"], AF.Square)
            mul("sq2", "s1", "s1")
            mul("q1", "sh1", "sh1")
            mul("t2", "q1", "s1")
            mul("p3c", "sq2", "q1")
            mul("p3s", "t2", "q1")
            mul("p5s", "s4", "q1")
            mul("p6s", "s4", "sq2")
            mul("p5c", "c4", "q1")
            mul("p6c", "c4", "sq2")
            mul("p7c", "c4", "p3c")
            mul("p7s", "s4", "p3c")
            mul("t8", "c4", "s4")
            mul("p9c", "c8", "q1")
            mul("p9s", "c8", "s1")
            mul("p10c", "c8", "sq2")
            mul("p10s", "c8", "t2")
            mul("p11c", "c8", "p3c")
            mul("p11s", "c8", "p3s")
            mul("p12c", "c8", "c4")
            mul("p12s", "t8", "c4")
            mul("t16", "c8", "t8")
            mul("p13c", "p12c", "q1")
            mul("p13s", "p12s", "q1")
            # last 4 tiles narrow per block: staggers y completion
            for j in js:
                slj = slice(j * JT, (j + 1) * JT)
                for name in NARROW:
                    a_, b_ = nfac[name]
                    nt = r5pool.tile([I, JT], F16, tag="nr", name=f"{name}_{j}")
                    nc.vector.tensor_tensor(nt[:], tiles[a_][:, slj],
                                            tiles[b_][:, slj], A.mult)
                    bi = SPEC_BASIS.index(name)
                    nc.tensor.matmul(
                        ys[j][:], wts[:, bi * O : (bi + 1) * O], nt[:],
                        start=False, stop=(bi == B - 1),
                    )
                    mm_count[j] += 1
                yo = r5pool.tile([O, JT], F32, tag="yo", name=f"yo{j}")
                if j == NJ - 1:
                    # last bank: DVE is idle by now; parallelizes final drain
                    nc.vector.tensor_copy(yo[:], ys[j][:])
                else:
                    nc.scalar.activation(yo[:], ys[j][:], AF.Copy)
                nc.gpsimd.dma_start(out=out_d[:, slj], in_=yo[:])
        assert all(c == B for c in mm_count), mm_count
    return nc


_NC_CACHE = {}


def kernel(x, cos_amplitudes, sin_amplitudes, bias, ln_gamma, ln_beta):
    from concourse.bass_utils import run_bass_kernel_spmd

    x = np.asarray(x, dtype=np.float32)
    ca = np.asarray(cos_amplitudes, dtype=np.float64)
    sa = np.asarray(sin_amplitudes, dtype=np.float64)
    bv = np.asarray(bias, dtype=np.float64)
    gv = np.asarray(ln_gamma, dtype=np.float32)
    be = np.asarray(ln_beta, dtype=np.float32)

    W = solve_weights(ca, sa, bv)
    w_all = np.ascontiguousarray(
        W.transpose(1, 0, 2).reshape(I, B * O)
    ).astype(np.float16)

    xT = np.ascontiguousarray(x.T)

    if 0 not in _NC_CACHE:
        _NC_CACHE[0] = build_device_program(False)
    nc = _NC_CACHE[0]

    in_maps = []
    for c in range(NCORES):
        in_maps.append(
            {
                "x_sh": np.ascontiguousarray(xT[:, c * NSH : (c + 1) * NSH]),
                "w_all": w_all,
            }
        )
    res = run_bass_kernel_spmd(nc, in_maps, list(range(NCORES)))
    outs = [res.results[c]["out_sh"] for c in range(NCORES)]
    full = np.concatenate(outs, axis=1)  # [O, N], centered over O
    y = np.ascontiguousarray(full.T).astype(np.float32)  # [N, O]
    var = np.mean(y * y, axis=1, keepdims=True)
    out = y * (1.0 / np.sqrt(var + EPS)) * gv[None, :] + be[None, :]
    return out.astype(np.float32)
